# revision 1
# baseline (speedup 1.0000x reference)
"""MoE (top-2, masked-dense reference) Trainium2 kernel, 8-core expert-parallel.

Strategy: each core owns one expert. The router (x @ Wr, softmax, top-2,
renormalized gates) is computed replicated on every core in fp32; each core
then runs its expert's FFN densely over all tokens in fp32r (TF32-like,
full-rate on the PE) and scales the transposed output y^T[d, t] by its
per-token gate (zero for unselected tokens). Partials [D, T] are summed with
an on-device ReduceScatter (split in two to overlap with compute); core c
returns the d-slice [128, T], which the host concatenates and transposes
back to [B, S, D].
"""
import numpy as np
import concourse.bass as bass
import concourse.mybir as mybir
import concourse.tile as tile
from concourse import bacc, bass_utils
from concourse.bass import ts, ds

B, S, D, FF, E = 4, 2048, 1024, 4096, 8
T = B * S                 # 8192 tokens
NCORES = 8
TB = 1024                 # tokens per FFN block
NB = T // TB              # 8 blocks
TC = 512                  # psum chunk (one bank)
DT = D // 128             # 8 d-tiles
FT = FF // 128            # 32 f-tiles
NTT = T // 128            # 64 token tiles
NQ = 4                    # f-quarters per block (8 ft each)
FQ = FT // NQ             # 8

AF = mybir.ActivationFunctionType
ALU = mybir.AluOpType


def build_nc():
    dt = mybir.dt
    f32, f32r = dt.float32, dt.float32r
    nc = bacc.Bacc("TRN2", target_bir_lowering=False, debug=False,
                   num_devices=NCORES)
    x_in = nc.dram_tensor("x", [T, D], f32, kind="ExternalInput").ap()
    wr_in = nc.dram_tensor("Wr", [D, E], f32, kind="ExternalInput").ap()
    w1_in = nc.dram_tensor("W1", [D, FF], f32, kind="ExternalInput").ap()
    b1_in = nc.dram_tensor("b1", [FF], f32, kind="ExternalInput").ap()
    w2_in = nc.dram_tensor("W2", [FF, D], f32, kind="ExternalInput").ap()
    b2_in = nc.dram_tensor("b2", [D], f32, kind="ExternalInput").ap()
    sel_in = nc.dram_tensor("sel", [128, E], f32, kind="ExternalInput").ap()
    id_in = nc.dram_tensor("ident", [128, 128], f32, kind="ExternalInput").ap()
    out_sh = nc.dram_tensor("out_shard", [128, T], f32, kind="ExternalOutput").ap()

    with tile.TileContext(nc) as tc:
        with tc.tile_pool(name="consts", bufs=1) as consts, \
             tc.tile_pool(name="sb", bufs=1) as sb, \
             tc.tile_pool(name="trps", bufs=2, space="PSUM") as trps, \
             tc.tile_pool(name="rps", bufs=2, space="PSUM") as rps, \
             tc.tile_pool(name="psH", bufs=1, space="PSUM") as psH, \
             tc.tile_pool(name="psY", bufs=1, space="PSUM") as psY, \
             tc.tile_pool(name="dram", bufs=1, space="DRAM") as dram:

            ident = consts.tile([128, 128], f32, name="ident")
            nc.sync.dma_start(ident[:], id_in[:])
            ones_row = consts.tile([1, 128], f32, name="ones_row")
            nc.vector.memset(ones_row[:], 1.0)
            wr_sb = consts.tile([128, DT, E], f32, name="wr_sb")
            nc.sync.dma_start(wr_sb[:], wr_in.rearrange("(dt p) e -> p dt e", p=128))
            b1f = consts.tile([128, FT], f32, name="b1f")
            nc.sync.dma_start(b1f[:], b1_in.rearrange("(ft p) -> p ft", p=128))
            b2c = consts.tile([128, DT], f32, name="b2c")
            nc.sync.dma_start(b2c[:], b2_in.rearrange("(dt p) -> p dt", p=128))
            sel_b = consts.tile([128, E], f32, name="sel_b")
            nc.sync.dma_start(sel_b[:], sel_in[:])

            w1r_d = dram.tile([128, DT, FF], f32r, name="w1r_d")
            w2r_d = dram.tile([128, FT, D], f32r, name="w2r_d")
            grow_d = dram.tile([1, T], f32, name="grow_d")
            xT_d = dram.tile([128, DT, T], f32r, name="xT_d")
            partial = dram.tile([4, D, T // 4], f32, name="partial")
            rs_outs = [dram.tile([128, T // 4], f32, name=f"rs_out{q}")
                       for q in range(4)]

            # ---- weight pre-round pass: fp32 -> f32r in DRAM ----
            _sid = nc.enter_named_scope("prepass", False)[0]
            for ch in range(FT):
                w1s = sb.tile([128, DT, 128], f32, name="w1s", bufs=1)
                nc.sync.dma_start(
                    w1s[:], w1_in[:, ds(ch * 128, 128)].rearrange(
                        "(dt p) f -> p dt f", p=128))
                w1rr = sb.tile([128, DT, 128], f32r, name="w1rr", bufs=1)
                nc.vector.tensor_copy(w1rr[:], w1s[:])
                nc.sync.dma_start(w1r_d[:, :, ds(ch * 128, 128)], w1rr[:])
            for ch in range(FT):
                w2s = sb.tile([128, D], f32, name="w2s", bufs=1)
                nc.sync.dma_start(w2s[:], w2_in[ts(ch, 128), :])
                w2rr = sb.tile([128, D], f32r, name="w2rr", bufs=1)
                nc.vector.tensor_copy(w2rr[:], w2s[:])
                nc.sync.dma_start(w2r_d[:, ch, :], w2rr[:])
            nc.leave_named_scope("prepass", _sid, False)

            # ---- router pass ----
            # 1) logits for all tokens -> logit_sb [128, NTT, E]
            _sid = nc.enter_named_scope("router", False)[0]
            logit_sb = sb.tile([128, NTT, E], f32, name="logit_sb")
            for tt in range(NTT):
                x_tile = sb.tile([128, D], f32, name="x_tile", bufs=2)
                nc.sync.dma_start(x_tile[:], x_in[ts(tt, 128), :])
                xtf = sb.tile([128, DT, 128], f32, name="xtf", bufs=2)
                xtr = sb.tile([128, DT, 128], f32r, name="xtr", bufs=1)
                for dti in range(DT):
                    tr = trps.tile([128, 128], f32, name="tr", tag="tr")
                    nc.tensor.transpose(tr[:], x_tile[:, ds(dti * 128, 128)], ident[:])
                    nc.scalar.copy(xtf[:, dti, :], tr[:])
                    nc.vector.tensor_copy(xtr[:, dti, :], tr[:])
                nc.sync.dma_start(xT_d[:, :, ts(tt, 128)], xtr[:])
                r_ps = rps.tile([128, E], f32, name="r_ps", tag="r_ps")
                for dti in range(DT):
                    nc.tensor.matmul(r_ps[:], xtf[:, dti, :], wr_sb[:, dti, :],
                                     start=(dti == 0), stop=(dti == DT - 1))
                nc.vector.tensor_copy(logit_sb[:, tt, :], r_ps[:])
            # 2) batched router math over [128, NTT, E]
            X3 = mybir.AxisListType.X
            lt = logit_sb

            def bcE(ap):   # [128, NTT, 1] -> broadcast over E
                return ap.broadcast_to([128, NTT, E])

            m1 = sb.tile([128, NTT, 1], f32, name="m1")
            nc.vector.reduce_max(m1[:], lt[:], axis=X3)
            eq1 = sb.tile([128, NTT, E], f32, name="eq1")
            nc.vector.tensor_tensor(eq1[:], lt[:], bcE(m1[:]), op=ALU.is_equal)
            l2 = sb.tile([128, NTT, E], f32, name="l2")
            nc.vector.tensor_scalar(l2[:], eq1[:], -1e30, None, op0=ALU.mult)
            nc.vector.tensor_tensor(l2[:], l2[:], lt[:], op=ALU.add)
            m2 = sb.tile([128, NTT, 1], f32, name="m2")
            nc.vector.reduce_max(m2[:], l2[:], axis=X3)
            eq2 = sb.tile([128, NTT, E], f32, name="eq2")
            nc.vector.tensor_tensor(eq2[:], l2[:], bcE(m2[:]), op=ALU.is_equal)
            m1n = sb.tile([128, NTT, 1], f32, name="m1n")
            nc.vector.tensor_scalar(m1n[:], m1[:], -1.0, None, op0=ALU.mult)
            sh = sb.tile([128, NTT, E], f32, name="sh", tag="l2")
            nc.vector.tensor_tensor(sh[:], lt[:], bcE(m1n[:]), op=ALU.add)
            ex = sb.tile([128, NTT, E], f32, name="ex")
            nc.scalar.activation(ex[:], sh[:], AF.Exp)
            z = sb.tile([128, NTT, 1], f32, name="z")
            nc.vector.reduce_sum(z[:], ex[:], axis=X3)
            rz = sb.tile([128, NTT, 1], f32, name="rz")
            nc.vector.reciprocal(rz[:], z[:])
            sh2 = sb.tile([128, NTT, 1], f32, name="sh2")
            nc.vector.tensor_tensor(sh2[:], m2[:], m1n[:], op=ALU.add)
            p2 = sb.tile([128, NTT, 1], f32, name="p2")
            nc.scalar.activation(p2[:], sh2[:], AF.Exp)
            nc.vector.tensor_tensor(p2[:], p2[:], rz[:], op=ALU.mult)
            ep1 = sb.tile([128, NTT, 1], f32, name="ep1")
            nc.scalar.activation(ep1[:], rz[:], AF.Exp)
            ep2 = sb.tile([128, NTT, 1], f32, name="ep2")
            nc.scalar.activation(ep2[:], p2[:], AF.Exp)
            s12 = sb.tile([128, NTT, 1], f32, name="s12")
            nc.vector.tensor_tensor(s12[:], ep1[:], ep2[:], op=ALU.add)
            rs12 = sb.tile([128, NTT, 1], f32, name="rs12")
            nc.vector.reciprocal(rs12[:], s12[:])
            g1 = sb.tile([128, NTT, 1], f32, name="g1")
            nc.vector.tensor_tensor(g1[:], ep1[:], rs12[:], op=ALU.mult)
            g2 = sb.tile([128, NTT, 1], f32, name="g2")
            nc.vector.tensor_tensor(g2[:], ep2[:], rs12[:], op=ALU.mult)
            G = sb.tile([128, NTT, E], f32, name="G")
            nc.vector.tensor_tensor(G[:], eq1[:], bcE(g1[:]), op=ALU.mult)
            nc.vector.tensor_tensor(eq2[:], eq2[:], bcE(g2[:]), op=ALU.mult)
            nc.vector.tensor_tensor(G[:], G[:], eq2[:], op=ALU.add)
            # 3) select this expert's gate column per token; transpose batched
            selb3 = sel_b[:].unsqueeze(1).broadcast_to([128, NTT, E])
            gsel = sb.tile([128, NTT, E], f32, name="gsel", tag="ex")
            nc.vector.tensor_tensor(gsel[:], G[:], selb3, op=ALU.mult)
            gcol = sb.tile([128, NTT, 1], f32, name="gcol")
            nc.vector.reduce_sum(gcol[:], gsel[:], axis=X3)
            gt_ps = trps.tile([NTT, 128], f32, name="gt_ps", tag="tr")
            nc.tensor.transpose(gt_ps[:], gcol[:, :, 0], ident[:])
            gt_sb = sb.tile([NTT, 128], f32, name="gt_sb")
            nc.scalar.copy(gt_sb[:], gt_ps[:])
            nc.sync.dma_start(
                grow_d[0:1, :].rearrange("o (tt p) -> (o tt) p", p=128), gt_sb[:])
            nc.leave_named_scope("router", _sid, False)

            # ---- FFN blocks ----
            xT = sb.tile([128, DT, TB], f32r, name="xT")
            hq = [sb.tile([128, FQ, TB], f32r, name=f"hq{q}") for q in range(2)]
            y_acc = sb.tile([128, DT, TB], f32, name="y_acc")
            for tb in range(NB):
                _sid = nc.enter_named_scope(f"block{tb}", False)[0]
                grow_blk = sb.tile([1, TB], f32, name="grow_blk", bufs=1)
                nc.sync.dma_start(grow_blk[:], grow_d[0:1, ts(tb, TB)])
                gbc_tiles = []
                for tci in range(TB // TC):
                    gbc = rps.tile([128, TC], f32, name=f"gbc{tci}", tag="r_ps")
                    nc.tensor.matmul(gbc[:], ones_row[:],
                                     grow_blk[0:1, ds(tci * TC, TC)],
                                     start=True, stop=True)
                    gbc_tiles.append(gbc)
                nc.sync.dma_start(xT[:], xT_d[:, :, ts(tb, TB)])
                for q in range(NQ):
                    h = hq[q % 2]
                    # GEMM1 quarter: h[:, fq, :] = gelu(xT @ W1 quarter + b1)
                    for fq in range(FQ):
                        ft = q * FQ + fq
                        w1r = sb.tile([128, DT, 128], f32r, name="w1r", bufs=2)
                        nc.sync.dma_start(w1r[:], w1r_d[:, :, ds(ft * 128, 128)])
                        phs = [psH.tile([128, TC], f32, name=f"ph{tci}")
                               for tci in range(TB // TC)]
                        for dti in range(DT):
                            for tci in range(TB // TC):
                                nc.tensor.matmul(
                                    phs[tci][:], w1r[:, dti, :],
                                    xT[:, dti, ds(tci * TC, TC)],
                                    start=(dti == 0), stop=(dti == DT - 1))
                        for tci in range(TB // TC):
                            nc.scalar.activation(
                                h[:, fq, ds(tci * TC, TC)], phs[tci][:], AF.Gelu,
                                bias=b1f[:, ft:ft + 1], scale=1.0)
                    # GEMM2 quarter: y_acc (+)= h @ W2 quarter
                    for dti in range(DT):
                        pys = [psY.tile([128, TC], f32, name=f"py{tci}")
                               for tci in range(TB // TC)]
                        w2r = sb.tile([128, FQ, 128], f32r, name="w2r", bufs=2)
                        nc.sync.dma_start(
                            w2r[:], w2r_d[:, ds(q * FQ, FQ), ds(dti * 128, 128)])
                        for j in range(FQ):
                            for tci in range(TB // TC):
                                nc.tensor.matmul(
                                    pys[tci][:], w2r[:, j, :],
                                    h[:, j, ds(tci * TC, TC)],
                                    start=(j == 0), stop=(j == FQ - 1))
                        for tci in range(TB // TC):
                            if q == 0:
                                nc.scalar.activation(
                                    y_acc[:, dti, ds(tci * TC, TC)], pys[tci][:],
                                    AF.Identity, bias=b2c[:, dti:dti + 1], scale=1.0)
                            elif q < NQ - 1:
                                nc.vector.tensor_tensor(
                                    y_acc[:, dti, ds(tci * TC, TC)],
                                    y_acc[:, dti, ds(tci * TC, TC)],
                                    pys[tci][:], op=ALU.add)
                            else:
                                yo = sb.tile([128, TC], f32, name="yo", bufs=2)
                                nc.vector.tensor_tensor(
                                    yo[:], pys[tci][:],
                                    y_acc[:, dti, ds(tci * TC, TC)], op=ALU.add)
                                nc.vector.tensor_tensor(
                                    yo[:], yo[:], gbc_tiles[tci][:], op=ALU.mult)
                                nc.sync.dma_start(
                                    partial[tb // 2,
                                            ds(dti * 128, 128),
                                            ds((tb % 2) * TB + tci * TC,
                                               TC)], yo[:])
                nc.leave_named_scope(f"block{tb}", _sid, False)
                if tb % 2 == 1 and tb < NB - 1:
                    q = tb // 2
                    _sid = nc.enter_named_scope(f"rs{q}", False)[0]
                    nc.gpsimd.collective_compute(
                        "ReduceScatter", ALU.add,
                        replica_groups=[list(range(NCORES))],
                        ins=[partial[q].opt()], outs=[rs_outs[q].opt()])
                    nc.sync.dma_start(out_sh[:, ds(q * (T // 4), T // 4)],
                                      rs_outs[q][:])
                    nc.leave_named_scope(f"rs{q}", _sid, False)

            _sid = nc.enter_named_scope("rs3", False)[0]
            nc.gpsimd.collective_compute(
                "ReduceScatter", ALU.add,
                replica_groups=[list(range(NCORES))],
                ins=[partial[3].opt()], outs=[rs_outs[3].opt()])
            nc.sync.dma_start(out_sh[:, ds(3 * (T // 4), T // 4)], rs_outs[3][:])
            nc.leave_named_scope("rs3", _sid, False)
    nc.compile()
    return nc


_NC_CACHE = None


def make_in_maps(x, W_router, W1, b1, W2, b2):
    x2d = np.ascontiguousarray(np.asarray(x, dtype=np.float32).reshape(T, D))
    Wr = np.ascontiguousarray(np.asarray(W_router, dtype=np.float32))
    W1 = np.asarray(W1, dtype=np.float32)
    b1 = np.asarray(b1, dtype=np.float32)
    W2 = np.asarray(W2, dtype=np.float32)
    b2 = np.asarray(b2, dtype=np.float32)
    ident = np.eye(128, dtype=np.float32)
    in_maps = []
    for c in range(NCORES):
        sel = np.zeros((128, E), dtype=np.float32)
        sel[:, c] = 1.0
        in_maps.append({
            "x": x2d, "Wr": Wr,
            "W1": np.ascontiguousarray(W1[c]),
            "b1": np.ascontiguousarray(b1[c]),
            "W2": np.ascontiguousarray(W2[c]),
            "b2": np.ascontiguousarray(b2[c]),
            "sel": sel,
            "ident": ident,
        })
    return in_maps


def kernel(x, W_router, W1, b1, W2, b2):
    global _NC_CACHE
    if _NC_CACHE is None:
        _NC_CACHE = build_nc()
    nc = _NC_CACHE
    in_maps = make_in_maps(x, W_router, W1, b1, W2, b2)
    res = bass_utils.run_bass_kernel_spmd(nc, in_maps,
                                          core_ids=list(range(NCORES)))
    yT = np.concatenate([res.results[c]["out_shard"] for c in range(NCORES)],
                        axis=0)          # [D, T]
    return np.ascontiguousarray(yT.T).reshape(B, S, D).astype(np.float32)



# revision 2
# speedup vs baseline: 5.0547x; 5.0547x over previous
"""MoE (top-2, masked-dense reference) Trainium2 kernel, 8-core, sparse.

Two launches:
  A) Router, data-parallel: core c computes logits/softmax/top-2/renormalized
     gates for tokens [c*1024, (c+1)*1024) in fp32 (same math as the
     reference), returning the gate matrix G [T, E] (gate value for the two
     selected experts per token, 0 elsewhere).
  B) FFN, expert-parallel: the host compacts the tokens routed to each
     expert (capacity C, padded with zeros), pre-transposes/casts them to
     bf16, and core e runs its expert's dense FFN (gelu(x@W1+b1)@W2+b2)*g
     over its C tokens only — 1/4 of the masked-dense FLOPs.  The host
     scatter-adds the two expert contributions per token back to [B, S, D]
     (unshard of the overlapping output sharding).
"""
import numpy as np
import ml_dtypes
import concourse.bass as bass
import concourse.mybir as mybir
import concourse.tile as tile
from concourse import bacc, bass_utils
from concourse.bass import ts, ds

B, S, D, FF, E = 4, 2048, 1024, 4096, 8
T = B * S                 # 8192 tokens
NCORES = 8
TPC = T // NCORES         # tokens per core in the router launch
DT = D // 128             # 8 d-tiles
FT = FF // 128            # 32 f-tiles
TB = 512                  # FFN token block (= one PSUM bank of fp32)
C_DEFAULT = 2560          # default per-expert token capacity (multiple of TB)

AF = mybir.ActivationFunctionType
ALU = mybir.AluOpType
BF16 = ml_dtypes.bfloat16


def build_router_nc():
    dt = mybir.dt
    f32 = dt.float32
    NTT = TPC // 128      # 8 token tiles per core
    nc = bacc.Bacc("TRN2", target_bir_lowering=False, debug=False,
                   num_devices=NCORES)
    x_in = nc.dram_tensor("x", [TPC, D], f32, kind="ExternalInput").ap()
    wr_in = nc.dram_tensor("Wr", [D, E], f32, kind="ExternalInput").ap()
    id_in = nc.dram_tensor("ident", [128, 128], f32, kind="ExternalInput").ap()
    g_out = nc.dram_tensor("G", [TPC, E], f32, kind="ExternalOutput").ap()

    with tile.TileContext(nc) as tc:
        with tc.tile_pool(name="consts", bufs=1) as consts, \
             tc.tile_pool(name="sb", bufs=1) as sb, \
             tc.tile_pool(name="trps", bufs=2, space="PSUM") as trps, \
             tc.tile_pool(name="rps", bufs=2, space="PSUM") as rps:
            ident = consts.tile([128, 128], f32, name="ident")
            nc.sync.dma_start(ident[:], id_in[:])
            wr_sb = consts.tile([128, DT, E], f32, name="wr_sb")
            nc.sync.dma_start(wr_sb[:], wr_in.rearrange("(dt p) e -> p dt e", p=128))

            logit_sb = sb.tile([128, NTT, E], f32, name="logit_sb")
            for tt in range(NTT):
                x_tile = sb.tile([128, D], f32, name="x_tile", bufs=2)
                nc.sync.dma_start(x_tile[:], x_in[ts(tt, 128), :])
                xtf = sb.tile([128, DT, 128], f32, name="xtf", bufs=2)
                for dti in range(DT):
                    tr = trps.tile([128, 128], f32, name="tr", tag="tr")
                    nc.tensor.transpose(tr[:], x_tile[:, ds(dti * 128, 128)], ident[:])
                    nc.scalar.copy(xtf[:, dti, :], tr[:])
                r_ps = rps.tile([128, E], f32, name="r_ps", tag="r_ps")
                for dti in range(DT):
                    nc.tensor.matmul(r_ps[:], xtf[:, dti, :], wr_sb[:, dti, :],
                                     start=(dti == 0), stop=(dti == DT - 1))
                nc.vector.tensor_copy(logit_sb[:, tt, :], r_ps[:])

            # batched router math over [128, NTT, E] (same as reference:
            # softmax over E, top-2, softmax-renormalize the 2 gates)
            X3 = mybir.AxisListType.X
            lt = logit_sb

            def bcE(ap):
                return ap.broadcast_to([128, NTT, E])

            m1 = sb.tile([128, NTT, 1], f32, name="m1")
            nc.vector.reduce_max(m1[:], lt[:], axis=X3)
            eq1 = sb.tile([128, NTT, E], f32, name="eq1")
            nc.vector.tensor_tensor(eq1[:], lt[:], bcE(m1[:]), op=ALU.is_equal)
            l2 = sb.tile([128, NTT, E], f32, name="l2")
            nc.vector.tensor_scalar(l2[:], eq1[:], -1e30, None, op0=ALU.mult)
            nc.vector.tensor_tensor(l2[:], l2[:], lt[:], op=ALU.add)
            m2 = sb.tile([128, NTT, 1], f32, name="m2")
            nc.vector.reduce_max(m2[:], l2[:], axis=X3)
            eq2 = sb.tile([128, NTT, E], f32, name="eq2")
            nc.vector.tensor_tensor(eq2[:], l2[:], bcE(m2[:]), op=ALU.is_equal)
            m1n = sb.tile([128, NTT, 1], f32, name="m1n")
            nc.vector.tensor_scalar(m1n[:], m1[:], -1.0, None, op0=ALU.mult)
            sh = sb.tile([128, NTT, E], f32, name="sh", tag="l2")
            nc.vector.tensor_tensor(sh[:], lt[:], bcE(m1n[:]), op=ALU.add)
            ex = sb.tile([128, NTT, E], f32, name="ex")
            nc.scalar.activation(ex[:], sh[:], AF.Exp)
            z = sb.tile([128, NTT, 1], f32, name="z")
            nc.vector.reduce_sum(z[:], ex[:], axis=X3)
            rz = sb.tile([128, NTT, 1], f32, name="rz")
            nc.vector.reciprocal(rz[:], z[:])
            sh2 = sb.tile([128, NTT, 1], f32, name="sh2")
            nc.vector.tensor_tensor(sh2[:], m2[:], m1n[:], op=ALU.add)
            p2 = sb.tile([128, NTT, 1], f32, name="p2")
            nc.scalar.activation(p2[:], sh2[:], AF.Exp)
            nc.vector.tensor_tensor(p2[:], p2[:], rz[:], op=ALU.mult)
            ep1 = sb.tile([128, NTT, 1], f32, name="ep1")
            nc.scalar.activation(ep1[:], rz[:], AF.Exp)
            ep2 = sb.tile([128, NTT, 1], f32, name="ep2")
            nc.scalar.activation(ep2[:], p2[:], AF.Exp)
            s12 = sb.tile([128, NTT, 1], f32, name="s12")
            nc.vector.tensor_tensor(s12[:], ep1[:], ep2[:], op=ALU.add)
            rs12 = sb.tile([128, NTT, 1], f32, name="rs12")
            nc.vector.reciprocal(rs12[:], s12[:])
            g1 = sb.tile([128, NTT, 1], f32, name="g1")
            nc.vector.tensor_tensor(g1[:], ep1[:], rs12[:], op=ALU.mult)
            g2 = sb.tile([128, NTT, 1], f32, name="g2")
            nc.vector.tensor_tensor(g2[:], ep2[:], rs12[:], op=ALU.mult)
            G = sb.tile([128, NTT, E], f32, name="G")
            nc.vector.tensor_tensor(G[:], eq1[:], bcE(g1[:]), op=ALU.mult)
            nc.vector.tensor_tensor(eq2[:], eq2[:], bcE(g2[:]), op=ALU.mult)
            nc.vector.tensor_tensor(G[:], G[:], eq2[:], op=ALU.add)
            nc.sync.dma_start(g_out.rearrange("(tt p) e -> p tt e", p=128), G[:])
    nc.compile()
    return nc


def build_ffn_nc(C):
    dt = mybir.dt
    f32, bf16 = dt.float32, dt.bfloat16
    NB = C // TB
    nc = bacc.Bacc("TRN2", target_bir_lowering=False, debug=False,
                   num_devices=NCORES)
    xgt_in = nc.dram_tensor("xgT", [128, DT, C], bf16, kind="ExternalInput").ap()
    w1_in = nc.dram_tensor("W1t", [128, DT, FF], bf16, kind="ExternalInput").ap()
    w2_in = nc.dram_tensor("W2t", [128, FT, D], bf16, kind="ExternalInput").ap()
    b1_in = nc.dram_tensor("b1", [FF], f32, kind="ExternalInput").ap()
    b2_in = nc.dram_tensor("b2", [D], f32, kind="ExternalInput").ap()
    gb_in = nc.dram_tensor("gb", [128, C], f32, kind="ExternalInput").ap()
    y_out = nc.dram_tensor("yT", [128, DT, C], f32, kind="ExternalOutput").ap()

    with tile.TileContext(nc) as tc:
        with tc.tile_pool(name="consts", bufs=1) as consts, \
             tc.tile_pool(name="sb", bufs=1) as sb, \
             tc.tile_pool(name="psH", bufs=2, space="PSUM") as psH, \
             tc.tile_pool(name="psY", bufs=2, space="PSUM") as psY:
            b1f = consts.tile([128, FT], f32, name="b1f")
            nc.sync.dma_start(b1f[:], b1_in.rearrange("(ft p) -> p ft", p=128))
            b2c = consts.tile([128, DT], f32, name="b2c")
            nc.sync.dma_start(b2c[:], b2_in.rearrange("(dt p) -> p dt", p=128))
            gb = consts.tile([128, C], f32, name="gb")
            nc.sync.dma_start(gb[:], gb_in[:])
            w1sb = consts.tile([128, DT, FF], bf16, name="w1sb")
            for q in range(8):
                nc.sync.dma_start(w1sb[:, :, ds(q * (FF // 8), FF // 8)],
                                  w1_in[:, :, ds(q * (FF // 8), FF // 8)])
            w2sb = consts.tile([128, FT, D], bf16, name="w2sb")
            for q in range(4):
                nc.sync.dma_start(w2sb[:, ds(q * (FT // 4), FT // 4), :],
                                  w2_in[:, ds(q * (FT // 4), FT // 4), :])

            for b in range(NB):
                xT = sb.tile([128, DT, TB], bf16, name="xT", bufs=2)
                nc.sync.dma_start(xT[:], xgt_in[:, :, ds(b * TB, TB)])
                h = sb.tile([128, FT, TB], bf16, name="h", bufs=1)
                for ft in range(FT):
                    ph = psH.tile([128, TB], f32, name="ph", tag="ph")
                    for dti in range(DT):
                        nc.tensor.matmul(ph[:], w1sb[:, dti, ds(ft * 128, 128)],
                                         xT[:, dti, :],
                                         start=(dti == 0), stop=(dti == DT - 1))
                    nc.scalar.activation(h[:, ft, :], ph[:], AF.Gelu,
                                         bias=b1f[:, ft:ft + 1], scale=1.0)
                for dti in range(DT):
                    py = psY.tile([128, TB], f32, name="py", tag="py")
                    for j in range(FT):
                        nc.tensor.matmul(py[:], w2sb[:, j, ds(dti * 128, 128)],
                                         h[:, j, :],
                                         start=(j == 0), stop=(j == FT - 1))
                    yo = sb.tile([128, TB], f32, name="yo", bufs=2)
                    nc.scalar.activation(yo[:], py[:], AF.Identity,
                                         bias=b2c[:, dti:dti + 1], scale=1.0)
                    nc.vector.tensor_tensor(yo[:], yo[:], gb[:, ds(b * TB, TB)],
                                            op=ALU.mult)
                    nc.sync.dma_start(y_out[:, dti, ds(b * TB, TB)], yo[:])
    nc.compile()
    return nc


_ROUTER_NC = None
_FFN_NCS = {}
last_runs = []            # [(name, nc, in_maps)] of the most recent kernel()


def _get_router_nc():
    global _ROUTER_NC
    if _ROUTER_NC is None:
        _ROUTER_NC = build_router_nc()
    return _ROUTER_NC


def _get_ffn_nc(C):
    if C not in _FFN_NCS:
        _FFN_NCS[C] = build_ffn_nc(C)
    return _FFN_NCS[C]


def kernel(x, W_router, W1, b1, W2, b2):
    global last_runs
    x2d = np.ascontiguousarray(np.asarray(x, np.float32).reshape(T, D))
    Wr = np.ascontiguousarray(np.asarray(W_router, np.float32))
    W1 = np.asarray(W1, np.float32)
    b1 = np.asarray(b1, np.float32)
    W2 = np.asarray(W2, np.float32)
    b2 = np.asarray(b2, np.float32)
    ident = np.eye(128, dtype=np.float32)

    # --- launch A: router ---
    ncA = _get_router_nc()
    in_maps_A = [{"x": x2d[c * TPC:(c + 1) * TPC], "Wr": Wr, "ident": ident}
                 for c in range(NCORES)]
    resA = bass_utils.run_bass_kernel_spmd(ncA, in_maps_A,
                                           core_ids=list(range(NCORES)))
    G = np.concatenate([resA.results[c]["G"] for c in range(NCORES)], axis=0)

    # --- host: compact tokens per expert ---
    idxs, gates, cnts = [], [], []
    for e in range(E):
        idx = np.nonzero(G[:, e] > 0.0)[0]
        idxs.append(idx)
        cnts.append(len(idx))
        gates.append(G[idx, e].astype(np.float32))
    maxc = max(cnts)
    C = max(C_DEFAULT, -(-maxc // TB) * TB)
    ncB = _get_ffn_nc(C)

    x2d_bf = x2d.astype(BF16)
    W1b = W1.astype(BF16)
    W2b = W2.astype(BF16)
    in_maps_B = []
    for e in range(E):
        xg = np.zeros((C, D), BF16)
        xg[:cnts[e]] = x2d_bf[idxs[e]]
        xgT = np.ascontiguousarray(
            xg.T.reshape(DT, 128, C).transpose(1, 0, 2))
        W1t = np.ascontiguousarray(
            W1b[e].reshape(DT, 128, FF).transpose(1, 0, 2))
        W2t = np.ascontiguousarray(
            W2b[e].reshape(FT, 128, D).transpose(1, 0, 2))
        g_pad = np.zeros(C, np.float32)
        g_pad[:cnts[e]] = gates[e]
        gb = np.ascontiguousarray(np.broadcast_to(g_pad, (128, C)))
        in_maps_B.append({"xgT": xgT, "W1t": W1t, "W2t": W2t,
                          "b1": np.ascontiguousarray(b1[e]),
                          "b2": np.ascontiguousarray(b2[e]),
                          "gb": gb})

    # --- launch B: expert FFNs ---
    resB = bass_utils.run_bass_kernel_spmd(ncB, in_maps_B,
                                           core_ids=list(range(NCORES)))

    last_runs = [("router", ncA, in_maps_A), ("ffn", ncB, in_maps_B)]

    # --- host: scatter-add the two expert contributions per token ---
    out = np.zeros((T, D), np.float32)
    for e in range(E):
        yT = resB.results[e]["yT"]                      # [128, DT, C]
        y = np.ascontiguousarray(yT.transpose(1, 0, 2)).reshape(D, C)
        out[idxs[e]] += y[:, :cnts[e]].T
    return out.reshape(B, S, D)


# revision 7
# speedup vs baseline: 5.4041x; 1.0691x over previous
"""MoE (top-2, masked-dense reference) Trainium2 kernel, 8-core, sparse.

Two launches:
  A) Router, data-parallel: core c computes logits/softmax/top-2/renormalized
     gates for tokens [c*1024, (c+1)*1024) in fp32 (same math as the
     reference), returning the gate matrix G [T, E] (gate value for the two
     selected experts per token, 0 elsewhere).
  B) FFN, expert-parallel: the host compacts the tokens routed to each
     expert (capacity C, padded with zeros), pre-transposes/casts them to
     bf16, and core e runs its expert's dense FFN (gelu(x@W1+b1)@W2+b2)*g
     over its C tokens only — 1/4 of the masked-dense FLOPs.  The host
     scatter-adds the two expert contributions per token back to [B, S, D]
     (unshard of the overlapping output sharding).
"""
import numpy as np
import ml_dtypes
import concourse.bass as bass
import concourse.mybir as mybir
import concourse.tile as tile
from concourse import bacc, bass_utils
from concourse.bass import ts, ds

B, S, D, FF, E = 4, 2048, 1024, 4096, 8
T = B * S                 # 8192 tokens
NCORES = 8
TPC = T // NCORES         # tokens per core in the router launch
DT = D // 128             # 8 d-tiles
FT = FF // 128            # 32 f-tiles
TBMAX = 1024              # FFN token block
TC = 512                  # psum chunk (one fp32 bank)

AF = mybir.ActivationFunctionType
ALU = mybir.AluOpType
BF16 = ml_dtypes.bfloat16


def build_router_nc():
    dt = mybir.dt
    f32 = dt.float32
    NTT = TPC // 128      # 8 token tiles per core
    nc = bacc.Bacc("TRN2", target_bir_lowering=False, debug=False,
                   num_devices=NCORES)
    x_in = nc.dram_tensor("x", [TPC, D], f32, kind="ExternalInput").ap()
    wr_in = nc.dram_tensor("Wr", [D, E], f32, kind="ExternalInput").ap()
    id_in = nc.dram_tensor("ident", [128, 128], f32, kind="ExternalInput").ap()
    g_out = nc.dram_tensor("G", [TPC, E], f32, kind="ExternalOutput").ap()

    with tile.TileContext(nc) as tc:
        with tc.tile_pool(name="consts", bufs=1) as consts, \
             tc.tile_pool(name="sb", bufs=1) as sb, \
             tc.tile_pool(name="trps", bufs=2, space="PSUM") as trps, \
             tc.tile_pool(name="rps", bufs=2, space="PSUM") as rps:
            ident = consts.tile([128, 128], f32, name="ident")
            nc.sync.dma_start(ident[:], id_in[:])
            wr_sb = consts.tile([128, DT, E], f32, name="wr_sb")
            nc.sync.dma_start(wr_sb[:], wr_in.rearrange("(dt p) e -> p dt e", p=128))

            # x^T tiles for the whole shard: [128, DT, TPC]
            xtf = sb.tile([128, DT, TPC], f32, name="xtf")
            for tt in range(NTT):
                x_tile = sb.tile([128, D], f32, name="x_tile", bufs=2)
                nc.sync.dma_start(x_tile[:], x_in[ts(tt, 128), :])
                for dti in range(DT):
                    tr = trps.tile([128, 128], f32, name="tr", tag="tr")
                    nc.tensor.transpose(tr[:], x_tile[:, ds(dti * 128, 128)], ident[:])
                    nc.scalar.copy(xtf[:, dti, ts(tt, 128)], tr[:])
            # logits^T [E, TPC] with Wr d-tiles stationary (cheap 8-col LDWs)
            ltT = sb.tile([8, TPC], f32, name="ltT")
            for ch in range(TPC // TC):
                lt_ps = rps.tile([8, TC], f32, name="lt_ps", tag="r_ps")
                for dti in range(DT):
                    nc.tensor.matmul(lt_ps[:], wr_sb[:, dti, :],
                                     xtf[:, dti, ds(ch * TC, TC)],
                                     start=(dti == 0), stop=(dti == DT - 1))
                nc.scalar.copy(ltT[:, ds(ch * TC, TC)], lt_ps[:])
            # transpose back to token-major [128, NTT, E]
            logit_sb = sb.tile([128, NTT, E], f32, name="logit_sb")
            for tt in range(NTT):
                bt_ps = trps.tile([128, E], f32, name="bt_ps", tag="tr")
                nc.tensor.matmul(bt_ps[:], ltT[:, ts(tt, 128)], ident[0:8, 0:E],
                                 start=True, stop=True)
                nc.vector.tensor_copy(logit_sb[:, tt, :], bt_ps[:])

            # batched router math over [128, NTT, E] (same as reference:
            # softmax over E, top-2, softmax-renormalize the 2 gates)
            X3 = mybir.AxisListType.X
            lt = logit_sb

            def bcE(ap):
                return ap.broadcast_to([128, NTT, E])

            m1 = sb.tile([128, NTT, 1], f32, name="m1")
            nc.vector.reduce_max(m1[:], lt[:], axis=X3)
            eq1 = sb.tile([128, NTT, E], f32, name="eq1")
            nc.vector.tensor_tensor(eq1[:], lt[:], bcE(m1[:]), op=ALU.is_equal)
            l2 = sb.tile([128, NTT, E], f32, name="l2")
            nc.vector.tensor_scalar(l2[:], eq1[:], -1e30, None, op0=ALU.mult)
            nc.vector.tensor_tensor(l2[:], l2[:], lt[:], op=ALU.add)
            m2 = sb.tile([128, NTT, 1], f32, name="m2")
            nc.vector.reduce_max(m2[:], l2[:], axis=X3)
            eq2 = sb.tile([128, NTT, E], f32, name="eq2")
            nc.vector.tensor_tensor(eq2[:], l2[:], bcE(m2[:]), op=ALU.is_equal)
            m1n = sb.tile([128, NTT, 1], f32, name="m1n")
            nc.vector.tensor_scalar(m1n[:], m1[:], -1.0, None, op0=ALU.mult)
            sh = sb.tile([128, NTT, E], f32, name="sh", tag="l2")
            nc.vector.tensor_tensor(sh[:], lt[:], bcE(m1n[:]), op=ALU.add)
            ex = sb.tile([128, NTT, E], f32, name="ex")
            nc.scalar.activation(ex[:], sh[:], AF.Exp)
            z = sb.tile([128, NTT, 1], f32, name="z")
            nc.vector.reduce_sum(z[:], ex[:], axis=X3)
            rz = sb.tile([128, NTT, 1], f32, name="rz")
            nc.vector.reciprocal(rz[:], z[:])
            sh2 = sb.tile([128, NTT, 1], f32, name="sh2")
            nc.vector.tensor_tensor(sh2[:], m2[:], m1n[:], op=ALU.add)
            p2 = sb.tile([128, NTT, 1], f32, name="p2")
            nc.scalar.activation(p2[:], sh2[:], AF.Exp)
            nc.vector.tensor_tensor(p2[:], p2[:], rz[:], op=ALU.mult)
            ep1 = sb.tile([128, NTT, 1], f32, name="ep1")
            nc.scalar.activation(ep1[:], rz[:], AF.Exp)
            ep2 = sb.tile([128, NTT, 1], f32, name="ep2")
            nc.scalar.activation(ep2[:], p2[:], AF.Exp)
            s12 = sb.tile([128, NTT, 1], f32, name="s12")
            nc.vector.tensor_tensor(s12[:], ep1[:], ep2[:], op=ALU.add)
            rs12 = sb.tile([128, NTT, 1], f32, name="rs12")
            nc.vector.reciprocal(rs12[:], s12[:])
            g1 = sb.tile([128, NTT, 1], f32, name="g1")
            nc.vector.tensor_tensor(g1[:], ep1[:], rs12[:], op=ALU.mult)
            g2 = sb.tile([128, NTT, 1], f32, name="g2")
            nc.vector.tensor_tensor(g2[:], ep2[:], rs12[:], op=ALU.mult)
            G = sb.tile([128, NTT, E], f32, name="G")
            nc.vector.tensor_tensor(G[:], eq1[:], bcE(g1[:]), op=ALU.mult)
            nc.vector.tensor_tensor(eq2[:], eq2[:], bcE(g2[:]), op=ALU.mult)
            nc.vector.tensor_tensor(G[:], G[:], eq2[:], op=ALU.add)
            nc.sync.dma_start(g_out.rearrange("(tt p) e -> p tt e", p=128), G[:])
    nc.compile()
    return nc


def ffn_blocks(C):
    blocks = [TBMAX] * (C // TBMAX)
    if C % TBMAX:
        blocks.append(C % TBMAX)
    return blocks


def build_ffn_nc(C):
    dt = mybir.dt
    f32, bf16 = dt.float32, dt.bfloat16
    assert C % 128 == 0
    nc = bacc.Bacc("TRN2", target_bir_lowering=False, debug=False,
                   num_devices=NCORES)
    xgt_in = nc.dram_tensor("xgT", [128, DT, C], bf16, kind="ExternalInput").ap()
    w1_in = nc.dram_tensor("W1t", [128, DT, FF], bf16, kind="ExternalInput").ap()
    w2_in = nc.dram_tensor("W2t", [128, FT, D], bf16, kind="ExternalInput").ap()
    b1_in = nc.dram_tensor("b1", [FF], f32, kind="ExternalInput").ap()
    b2_in = nc.dram_tensor("b2", [D], f32, kind="ExternalInput").ap()
    gb_in = nc.dram_tensor("gb", [128, C], f32, kind="ExternalInput").ap()
    y_out = nc.dram_tensor("yT", [128, DT, C], f32, kind="ExternalOutput").ap()

    with tile.TileContext(nc) as tc:
        with tc.tile_pool(name="consts", bufs=1) as consts, \
             tc.tile_pool(name="sb", bufs=1) as sb, \
             tc.tile_pool(name="psH", bufs=2, space="PSUM") as psH, \
             tc.tile_pool(name="psY", bufs=2, space="PSUM") as psY:
            b1f = consts.tile([128, FT], f32, name="b1f")
            nc.sync.dma_start(b1f[:], b1_in.rearrange("(ft p) -> p ft", p=128))
            b2c = consts.tile([128, DT], f32, name="b2c")
            nc.sync.dma_start(b2c[:], b2_in.rearrange("(dt p) -> p dt", p=128))
            gb = consts.tile([128, C], f32, name="gb")
            nc.sync.dma_start(gb[:], gb_in[:])
            w1sb = consts.tile([128, DT, FF], bf16, name="w1sb")
            for q in range(8):
                nc.sync.dma_start(w1sb[:, :, ds(q * (FF // 8), FF // 8)],
                                  w1_in[:, :, ds(q * (FF // 8), FF // 8)])

            pos = 0
            for b, TB in enumerate(ffn_blocks(C)):
                nch = -(-TB // TC)
                chs = [ds(pos + ci * TC, min(TC, TB - ci * TC)) for ci in range(nch)]
                lchs = [ds(ci * TC, min(TC, TB - ci * TC)) for ci in range(nch)]
                xT = sb.tile([128, DT, TB], bf16, name="xT", bufs=2)
                nc.sync.dma_start(xT[:], xgt_in[:, :, ds(pos, TB)])
                h = sb.tile([128, FT, TB], bf16, name="h", bufs=1)
                for ft in range(FT):
                    phs = [psH.tile([128, ch.size], f32, name=f"ph{ci}", tag=f"ph{ci}")
                           for ci, ch in enumerate(lchs)]
                    for dti in range(DT):
                        for ci in range(nch):
                            nc.tensor.matmul(phs[ci][:],
                                             w1sb[:, dti, ds(ft * 128, 128)],
                                             xT[:, dti, lchs[ci]],
                                             start=(dti == 0), stop=(dti == DT - 1))
                    for ci in range(nch):
                        nc.scalar.activation(h[:, ft, lchs[ci]], phs[ci][:], AF.Gelu,
                                             bias=b1f[:, ft:ft + 1], scale=1.0)
                for dti in range(DT):
                    w2t = sb.tile([128, FT, 128], bf16, name="w2t", bufs=2)
                    nc.sync.dma_start(w2t[:], w2_in[:, :, ds(dti * 128, 128)])
                    pys = [psY.tile([128, ch.size], f32, name=f"py{ci}", tag=f"py{ci}")
                           for ci, ch in enumerate(lchs)]
                    for j in range(FT):
                        for ci in range(nch):
                            nc.tensor.matmul(pys[ci][:], w2t[:, j, :],
                                             h[:, j, lchs[ci]],
                                             start=(j == 0), stop=(j == FT - 1))
                    for ci in range(nch):
                        yo = sb.tile([128, lchs[ci].size], f32, name="yo", bufs=4)
                        nc.scalar.activation(yo[:], pys[ci][:], AF.Identity,
                                             bias=b2c[:, dti:dti + 1], scale=1.0)
                        nc.vector.tensor_tensor(yo[:], yo[:], gb[:, chs[ci]],
                                                op=ALU.mult)
                        nc.sync.dma_start(y_out[:, dti, chs[ci]], yo[:])
                pos += TB
    nc.compile()
    return nc


_ROUTER_NC = None
_FFN_NCS = {}
last_runs = []            # [(name, nc, in_maps)] of the most recent kernel()


def _get_router_nc():
    global _ROUTER_NC
    if _ROUTER_NC is None:
        _ROUTER_NC = build_router_nc()
    return _ROUTER_NC


def _get_ffn_nc(C):
    if C not in _FFN_NCS:
        _FFN_NCS[C] = build_ffn_nc(C)
    return _FFN_NCS[C]


def kernel(x, W_router, W1, b1, W2, b2):
    global last_runs
    x2d = np.ascontiguousarray(np.asarray(x, np.float32).reshape(T, D))
    Wr = np.ascontiguousarray(np.asarray(W_router, np.float32))
    W1 = np.asarray(W1, np.float32)
    b1 = np.asarray(b1, np.float32)
    W2 = np.asarray(W2, np.float32)
    b2 = np.asarray(b2, np.float32)
    ident = np.eye(128, dtype=np.float32)

    # --- launch A: router ---
    ncA = _get_router_nc()
    in_maps_A = [{"x": x2d[c * TPC:(c + 1) * TPC], "Wr": Wr, "ident": ident}
                 for c in range(NCORES)]
    resA = bass_utils.run_bass_kernel_spmd(ncA, in_maps_A,
                                           core_ids=list(range(NCORES)))
    G = np.concatenate([resA.results[c]["G"] for c in range(NCORES)], axis=0)

    # --- host: compact tokens per expert ---
    idxs, gates, cnts = [], [], []
    for e in range(E):
        idx = np.nonzero(G[:, e] > 0.0)[0]
        idxs.append(idx)
        cnts.append(len(idx))
        gates.append(G[idx, e].astype(np.float32))
    maxc = max(cnts)
    C = max(1024, -(-maxc // 128) * 128)
    ncB = _get_ffn_nc(C)

    x2d_bf = x2d.astype(BF16)
    W1b = W1.astype(BF16)
    W2b = W2.astype(BF16)
    in_maps_B = []
    for e in range(E):
        xg = np.zeros((C, D), BF16)
        xg[:cnts[e]] = x2d_bf[idxs[e]]
        xgT = np.ascontiguousarray(
            xg.T.reshape(DT, 128, C).transpose(1, 0, 2))
        W1t = np.ascontiguousarray(
            W1b[e].reshape(DT, 128, FF).transpose(1, 0, 2))
        W2t = np.ascontiguousarray(
            W2b[e].reshape(FT, 128, D).transpose(1, 0, 2))
        g_pad = np.zeros(C, np.float32)
        g_pad[:cnts[e]] = gates[e]
        gb = np.ascontiguousarray(np.broadcast_to(g_pad, (128, C)))
        in_maps_B.append({"xgT": xgT, "W1t": W1t, "W2t": W2t,
                          "b1": np.ascontiguousarray(b1[e]),
                          "b2": np.ascontiguousarray(b2[e]),
                          "gb": gb})

    # --- launch B: expert FFNs ---
    resB = bass_utils.run_bass_kernel_spmd(ncB, in_maps_B,
                                           core_ids=list(range(NCORES)))

    last_runs = [("router", ncA, in_maps_A), ("ffn", ncB, in_maps_B)]

    # --- host: scatter-add the two expert contributions per token ---
    out = np.zeros((T, D), np.float32)
    for e in range(E):
        yT = resB.results[e]["yT"]                      # [128, DT, C]
        y = np.ascontiguousarray(yT.transpose(1, 0, 2)).reshape(D, C)
        out[idxs[e]] += y[:, :cnts[e]].T
    return out.reshape(B, S, D)


# revision 9
# speedup vs baseline: 5.4538x; 1.0092x over previous
"""MoE (top-2, masked-dense reference) Trainium2 kernel, 8-core, sparse.

Two launches:
  A) Router, data-parallel: core c computes logits/softmax/top-2/renormalized
     gates for tokens [c*1024, (c+1)*1024) in fp32 (same math as the
     reference), returning the gate matrix G [T, E] (gate value for the two
     selected experts per token, 0 elsewhere).
  B) FFN, expert-parallel: the host compacts the tokens routed to each
     expert (capacity C, padded with zeros), pre-transposes/casts them to
     bf16, and core e runs its expert's dense FFN (gelu(x@W1+b1)@W2+b2)*g
     over its C tokens only — 1/4 of the masked-dense FLOPs.  The host
     scatter-adds the two expert contributions per token back to [B, S, D]
     (unshard of the overlapping output sharding).
"""
import numpy as np
import ml_dtypes
import concourse.bass as bass
import concourse.mybir as mybir
import concourse.tile as tile
from concourse import bacc, bass_utils
from concourse.bass import ts, ds

B, S, D, FF, E = 4, 2048, 1024, 4096, 8
T = B * S                 # 8192 tokens
NCORES = 8
TPC = T // NCORES         # tokens per core in the router launch
DT = D // 128             # 8 d-tiles
FT = FF // 128            # 32 f-tiles
TBMAX = 1024              # FFN token block
TC = 512                  # psum chunk (one fp32 bank)

AF = mybir.ActivationFunctionType
ALU = mybir.AluOpType
BF16 = ml_dtypes.bfloat16


def build_router_nc():
    dt = mybir.dt
    f32 = dt.float32
    NTT = TPC // 128      # 8 token tiles per core
    nc = bacc.Bacc("TRN2", target_bir_lowering=False, debug=False,
                   num_devices=NCORES)
    x_in = nc.dram_tensor("x", [TPC, D], f32, kind="ExternalInput").ap()
    wr_in = nc.dram_tensor("Wr", [D, E], f32, kind="ExternalInput").ap()
    id_in = nc.dram_tensor("ident", [128, 128], f32, kind="ExternalInput").ap()
    g_out = nc.dram_tensor("G", [TPC, E], f32, kind="ExternalOutput").ap()

    with tile.TileContext(nc) as tc:
        with tc.tile_pool(name="consts", bufs=1) as consts, \
             tc.tile_pool(name="sb", bufs=1) as sb, \
             tc.tile_pool(name="trps", bufs=2, space="PSUM") as trps, \
             tc.tile_pool(name="rps", bufs=2, space="PSUM") as rps:
            ident = consts.tile([128, 128], f32, name="ident")
            nc.sync.dma_start(ident[:], id_in[:])
            wr_sb = consts.tile([128, DT, E], f32, name="wr_sb")
            nc.sync.dma_start(wr_sb[:], wr_in.rearrange("(dt p) e -> p dt e", p=128))

            # x^T tiles for the whole shard: [128, DT, TPC].  The DMA loads
            # each x tile with its 32x32 blocks position-swapped (partition
            # group <-> free group); the DVE stream-transpose then flips each
            # 32x32 block in place, yielding a full 128x128 transpose without
            # touching the PE.
            x_sw = x_in.rearrange(
                "(tt ja ap) (dt jb bp) -> tt jb ap dt ja bp",
                tt=NTT, ja=4, ap=32, dt=DT, jb=4, bp=32)
            xtf = sb.tile([128, DT, TPC], f32, name="xtf")
            for tt in range(NTT):
                xs = sb.tile([128, DT, 128], f32, name="xs", bufs=2)
                for jb in range(4):
                    dst = xs[:][ds(jb * 32, 32)].rearrange(
                        "p dt (ja bp) -> p dt ja bp", ja=4, bp=32)
                    nc.sync.dma_start(dst, x_sw[tt, jb])
                nc.vector.transpose(xtf[:, :, ts(tt, 128)], xs[:])
            # logits^T [E, TPC] with Wr d-tiles stationary (cheap 8-col LDWs)
            ltT = sb.tile([8, TPC], f32, name="ltT")
            for ch in range(TPC // TC):
                lt_ps = rps.tile([8, TC], f32, name="lt_ps", tag="r_ps")
                for dti in range(DT):
                    nc.tensor.matmul(lt_ps[:], wr_sb[:, dti, :],
                                     xtf[:, dti, ds(ch * TC, TC)],
                                     start=(dti == 0), stop=(dti == DT - 1))
                nc.scalar.copy(ltT[:, ds(ch * TC, TC)], lt_ps[:])
            # transpose back to token-major [128, NTT, E]
            logit_sb = sb.tile([128, NTT, E], f32, name="logit_sb")
            for tt in range(NTT):
                bt_ps = trps.tile([128, E], f32, name="bt_ps", tag="tr")
                nc.tensor.matmul(bt_ps[:], ltT[:, ts(tt, 128)], ident[0:8, 0:E],
                                 start=True, stop=True)
                nc.vector.tensor_copy(logit_sb[:, tt, :], bt_ps[:])

            # batched router math over [128, NTT, E] (same as reference:
            # softmax over E, top-2, softmax-renormalize the 2 gates)
            X3 = mybir.AxisListType.X
            lt = logit_sb

            def bcE(ap):
                return ap.broadcast_to([128, NTT, E])

            m1 = sb.tile([128, NTT, 1], f32, name="m1")
            nc.vector.reduce_max(m1[:], lt[:], axis=X3)
            eq1 = sb.tile([128, NTT, E], f32, name="eq1")
            nc.vector.tensor_tensor(eq1[:], lt[:], bcE(m1[:]), op=ALU.is_equal)
            l2 = sb.tile([128, NTT, E], f32, name="l2")
            nc.vector.tensor_scalar(l2[:], eq1[:], -1e30, None, op0=ALU.mult)
            nc.vector.tensor_tensor(l2[:], l2[:], lt[:], op=ALU.add)
            m2 = sb.tile([128, NTT, 1], f32, name="m2")
            nc.vector.reduce_max(m2[:], l2[:], axis=X3)
            eq2 = sb.tile([128, NTT, E], f32, name="eq2")
            nc.vector.tensor_tensor(eq2[:], l2[:], bcE(m2[:]), op=ALU.is_equal)
            m1n = sb.tile([128, NTT, 1], f32, name="m1n")
            nc.vector.tensor_scalar(m1n[:], m1[:], -1.0, None, op0=ALU.mult)
            sh = sb.tile([128, NTT, E], f32, name="sh", tag="l2")
            nc.vector.tensor_tensor(sh[:], lt[:], bcE(m1n[:]), op=ALU.add)
            ex = sb.tile([128, NTT, E], f32, name="ex")
            nc.scalar.activation(ex[:], sh[:], AF.Exp)
            z = sb.tile([128, NTT, 1], f32, name="z")
            nc.vector.reduce_sum(z[:], ex[:], axis=X3)
            rz = sb.tile([128, NTT, 1], f32, name="rz")
            nc.vector.reciprocal(rz[:], z[:])
            sh2 = sb.tile([128, NTT, 1], f32, name="sh2")
            nc.vector.tensor_tensor(sh2[:], m2[:], m1n[:], op=ALU.add)
            p2 = sb.tile([128, NTT, 1], f32, name="p2")
            nc.scalar.activation(p2[:], sh2[:], AF.Exp)
            nc.vector.tensor_tensor(p2[:], p2[:], rz[:], op=ALU.mult)
            ep1 = sb.tile([128, NTT, 1], f32, name="ep1")
            nc.scalar.activation(ep1[:], rz[:], AF.Exp)
            ep2 = sb.tile([128, NTT, 1], f32, name="ep2")
            nc.scalar.activation(ep2[:], p2[:], AF.Exp)
            s12 = sb.tile([128, NTT, 1], f32, name="s12")
            nc.vector.tensor_tensor(s12[:], ep1[:], ep2[:], op=ALU.add)
            rs12 = sb.tile([128, NTT, 1], f32, name="rs12")
            nc.vector.reciprocal(rs12[:], s12[:])
            g1 = sb.tile([128, NTT, 1], f32, name="g1")
            nc.vector.tensor_tensor(g1[:], ep1[:], rs12[:], op=ALU.mult)
            g2 = sb.tile([128, NTT, 1], f32, name="g2")
            nc.vector.tensor_tensor(g2[:], ep2[:], rs12[:], op=ALU.mult)
            G = sb.tile([128, NTT, E], f32, name="G")
            nc.vector.tensor_tensor(G[:], eq1[:], bcE(g1[:]), op=ALU.mult)
            nc.vector.tensor_tensor(eq2[:], eq2[:], bcE(g2[:]), op=ALU.mult)
            nc.vector.tensor_tensor(G[:], G[:], eq2[:], op=ALU.add)
            nc.sync.dma_start(g_out.rearrange("(tt p) e -> p tt e", p=128), G[:])
    nc.compile()
    return nc


def ffn_blocks(C):
    blocks = [TBMAX] * (C // TBMAX)
    if C % TBMAX:
        blocks.append(C % TBMAX)
    return blocks


def build_ffn_nc(C):
    dt = mybir.dt
    f32, bf16 = dt.float32, dt.bfloat16
    assert C % 128 == 0
    nc = bacc.Bacc("TRN2", target_bir_lowering=False, debug=False,
                   num_devices=NCORES)
    xgt_in = nc.dram_tensor("xgT", [128, DT, C], bf16, kind="ExternalInput").ap()
    w1_in = nc.dram_tensor("W1t", [128, DT, FF], bf16, kind="ExternalInput").ap()
    w2_in = nc.dram_tensor("W2t", [128, FT, D], bf16, kind="ExternalInput").ap()
    b1_in = nc.dram_tensor("b1", [FF], f32, kind="ExternalInput").ap()
    b2_in = nc.dram_tensor("b2", [D], f32, kind="ExternalInput").ap()
    gb_in = nc.dram_tensor("gb", [128, C], f32, kind="ExternalInput").ap()
    y_out = nc.dram_tensor("yT", [128, DT, C], f32, kind="ExternalOutput").ap()

    with tile.TileContext(nc) as tc:
        with tc.tile_pool(name="consts", bufs=1) as consts, \
             tc.tile_pool(name="sb", bufs=1) as sb, \
             tc.tile_pool(name="psH", bufs=2, space="PSUM") as psH, \
             tc.tile_pool(name="psY", bufs=2, space="PSUM") as psY:
            b1f = consts.tile([128, FT], f32, name="b1f")
            nc.sync.dma_start(b1f[:], b1_in.rearrange("(ft p) -> p ft", p=128))
            b2c = consts.tile([128, DT], f32, name="b2c")
            nc.sync.dma_start(b2c[:], b2_in.rearrange("(dt p) -> p dt", p=128))
            gb = consts.tile([128, C], f32, name="gb")
            nc.sync.dma_start(gb[:], gb_in[:])
            w1sb = consts.tile([128, DT, FF], bf16, name="w1sb")
            for q in range(8):
                nc.sync.dma_start(w1sb[:, :, ds(q * (FF // 8), FF // 8)],
                                  w1_in[:, :, ds(q * (FF // 8), FF // 8)])

            pos = 0
            for b, TB in enumerate(ffn_blocks(C)):
                nch = -(-TB // TC)
                chs = [ds(pos + ci * TC, min(TC, TB - ci * TC)) for ci in range(nch)]
                lchs = [ds(ci * TC, min(TC, TB - ci * TC)) for ci in range(nch)]
                xT = sb.tile([128, DT, TB], bf16, name="xT", bufs=2)
                nc.sync.dma_start(xT[:], xgt_in[:, :, ds(pos, TB)])
                h = sb.tile([128, FT, TB], bf16, name="h", bufs=1)
                for ft in range(FT):
                    phs = [psH.tile([128, ch.size], f32, name=f"ph{ci}", tag=f"ph{ci}")
                           for ci, ch in enumerate(lchs)]
                    for dti in range(DT):
                        for ci in range(nch):
                            nc.tensor.matmul(phs[ci][:],
                                             w1sb[:, dti, ds(ft * 128, 128)],
                                             xT[:, dti, lchs[ci]],
                                             start=(dti == 0), stop=(dti == DT - 1))
                    for ci in range(nch):
                        nc.scalar.activation(h[:, ft, lchs[ci]], phs[ci][:], AF.Gelu,
                                             bias=b1f[:, ft:ft + 1], scale=1.0)
                for dti in range(DT):
                    w2t = sb.tile([128, FT, 128], bf16, name="w2t", bufs=2)
                    nc.sync.dma_start(w2t[:], w2_in[:, :, ds(dti * 128, 128)])
                    pys = [psY.tile([128, ch.size], f32, name=f"py{ci}", tag=f"py{ci}")
                           for ci, ch in enumerate(lchs)]
                    for j in range(FT):
                        for ci in range(nch):
                            nc.tensor.matmul(pys[ci][:], w2t[:, j, :],
                                             h[:, j, lchs[ci]],
                                             start=(j == 0), stop=(j == FT - 1))
                    for ci in range(nch):
                        yo = sb.tile([128, lchs[ci].size], f32, name="yo", bufs=4)
                        nc.scalar.activation(yo[:], pys[ci][:], AF.Identity,
                                             bias=b2c[:, dti:dti + 1], scale=1.0)
                        nc.vector.tensor_tensor(yo[:], yo[:], gb[:, chs[ci]],
                                                op=ALU.mult)
                        nc.sync.dma_start(y_out[:, dti, chs[ci]], yo[:])
                pos += TB
    nc.compile()
    return nc


_ROUTER_NC = None
_FFN_NCS = {}
last_runs = []            # [(name, nc, in_maps)] of the most recent kernel()


def _get_router_nc():
    global _ROUTER_NC
    if _ROUTER_NC is None:
        _ROUTER_NC = build_router_nc()
    return _ROUTER_NC


def _get_ffn_nc(C):
    if C not in _FFN_NCS:
        _FFN_NCS[C] = build_ffn_nc(C)
    return _FFN_NCS[C]


def kernel(x, W_router, W1, b1, W2, b2):
    global last_runs
    x2d = np.ascontiguousarray(np.asarray(x, np.float32).reshape(T, D))
    Wr = np.ascontiguousarray(np.asarray(W_router, np.float32))
    W1 = np.asarray(W1, np.float32)
    b1 = np.asarray(b1, np.float32)
    W2 = np.asarray(W2, np.float32)
    b2 = np.asarray(b2, np.float32)
    ident = np.eye(128, dtype=np.float32)

    # --- launch A: router ---
    ncA = _get_router_nc()
    in_maps_A = [{"x": x2d[c * TPC:(c + 1) * TPC], "Wr": Wr, "ident": ident}
                 for c in range(NCORES)]
    resA = bass_utils.run_bass_kernel_spmd(ncA, in_maps_A,
                                           core_ids=list(range(NCORES)))
    G = np.concatenate([resA.results[c]["G"] for c in range(NCORES)], axis=0)

    # --- host: compact tokens per expert ---
    idxs, gates, cnts = [], [], []
    for e in range(E):
        idx = np.nonzero(G[:, e] > 0.0)[0]
        idxs.append(idx)
        cnts.append(len(idx))
        gates.append(G[idx, e].astype(np.float32))
    maxc = max(cnts)
    C = max(1024, -(-maxc // 128) * 128)
    ncB = _get_ffn_nc(C)

    x2d_bf = x2d.astype(BF16)
    W1b = W1.astype(BF16)
    W2b = W2.astype(BF16)
    in_maps_B = []
    for e in range(E):
        xg = np.zeros((C, D), BF16)
        xg[:cnts[e]] = x2d_bf[idxs[e]]
        xgT = np.ascontiguousarray(
            xg.T.reshape(DT, 128, C).transpose(1, 0, 2))
        W1t = np.ascontiguousarray(
            W1b[e].reshape(DT, 128, FF).transpose(1, 0, 2))
        W2t = np.ascontiguousarray(
            W2b[e].reshape(FT, 128, D).transpose(1, 0, 2))
        g_pad = np.zeros(C, np.float32)
        g_pad[:cnts[e]] = gates[e]
        gb = np.ascontiguousarray(np.broadcast_to(g_pad, (128, C)))
        in_maps_B.append({"xgT": xgT, "W1t": W1t, "W2t": W2t,
                          "b1": np.ascontiguousarray(b1[e]),
                          "b2": np.ascontiguousarray(b2[e]),
                          "gb": gb})

    # --- launch B: expert FFNs ---
    resB = bass_utils.run_bass_kernel_spmd(ncB, in_maps_B,
                                           core_ids=list(range(NCORES)))

    last_runs = [("router", ncA, in_maps_A), ("ffn", ncB, in_maps_B)]

    # --- host: scatter-add the two expert contributions per token ---
    out = np.zeros((T, D), np.float32)
    for e in range(E):
        yT = resB.results[e]["yT"]                      # [128, DT, C]
        y = np.ascontiguousarray(yT.transpose(1, 0, 2)).reshape(D, C)
        out[idxs[e]] += y[:, :cnts[e]].T
    return out.reshape(B, S, D)
